# revision 16
# baseline (speedup 1.0000x reference)
"""Trainium2 Bass kernel for nn_BinGATConv (2-layer GAT + LN + mean-pool + MLP).

Strategy (8 NeuronCores, SPMD), v2:
  - Nodes dst-sharded: core c owns dst nodes [c*5000, (c+1)*5000); edges are
    1D-partitioned by dst on the host and sorted by (dst_block, src_half).
  - 4 sequential SPMD launches; the host only reshards/concats/gathers
    index-addressed STAGED VALUES between them (halo exchange):
      P0: per-core slice of the L1 gather table T1[n] = [h1(n)|1] (bf16)
          plus per-node attention scalars s1(n), d1(n) (f32).
      P1: L1 message passing + ReLU/LN + W2 projection -> T2 slice + s2/d2
      P2: L2 message passing + ReLU/LN + per-graph partial mean-pool
      P3: combine 8 partial pools + tiny MLP head (replicated)
  - Between launches the host gathers per-edge score columns
    scol[e] = s(src_e), ccol[e] = d(dst_e)  (pure indexed data movement).
  - On device per 128-edge tile:
      ex   = exp(lrelu(scol+ccol))            (one wide DVE op + one ACT exp
                                               for the entire phase)
      M    = (iota == dstloc_e) * ex_e        (one DVE tensor_scalar, bf16 4x)
      PSUM[d, 0:F+1] += M^T @ [h[src_e] | 1]  (one PE matmul per tile)
  - Per-dst-block postproc: softmax divide + ReLU + LayerNorm with the
    rsqrt computed as exp(-0.5*ln(var+eps)) so every ACT function of the
    phase lives in ONE activation table (no LoadActFuncSet thrash).
"""

import re
from contextlib import ExitStack

import ml_dtypes
import numpy as np

import concourse.bass as bass
import concourse.bacc as bacc
import concourse.mybir as mybir
import concourse.tile as tile
import concourse.dve_ops as dvo
from concourse.dve_spec import Spec, Src0, Src1, C0, C1, C2, eq, maxx, select, Idx, relu, sq
from concourse.bass_utils import run_bass_kernel_spmd

F32 = mybir.dt.float32
BF16 = mybir.dt.bfloat16
I16 = mybir.dt.int16
NPBF = ml_dtypes.bfloat16

NCORES = 8
N = 40000
E = 640000
G = 64
SL = N // NCORES          # 5000 nodes per core
SLP = 5120                # padded slice (40*128)
NB = SLP // 128           # 40 dst blocks per core
LOROWS = 4 * SLP          # 20480 rows in each table half
F1 = 128                  # layer-1 feature dim
F2 = 64                   # layer-2 feature dim
ROW1 = 128                # bf16 cols per T1 row (256B): [h1(128)]
ROW2 = 128                # bf16 cols per T2 row (256B): [h2(64) | 1 | pad]
GRP = 4                   # dst blocks per gather group
EPS = 1e-5

_OPS = {}
TRACE = False
LAST_EXEC_NS = 0
EXEC_NS = []
_RUN = run_bass_kernel_spmd


def _register_ops():
    if "GAT_SCORE_ANT" in dvo._SUB_OPCODE_FOR_NAME:
        for key, name in (("score", "GAT_SCORE_ANT"), ("var", "GAT_VAR_ANT"),
                          ("hb", "GAT_HB_ANT")):
            _OPS[key] = next(o for o in dvo.OPS if o.name == name)
        return

    def score_ref(in0, in1, s0, s1, imm2):
        a0 = np.asarray(in0, np.float32).reshape(np.asarray(in0).shape[0], -1)
        a1 = np.asarray(in1, np.float32).reshape(np.asarray(in1).shape[0], -1)
        t = a0 + a1
        return np.maximum(t, t * imm2).astype(np.float32)

    def var_ref(in0, in1, s0, s1, imm2):
        a0 = np.asarray(in0, np.float32).reshape(np.asarray(in0).shape[0], -1)
        a1 = np.asarray(in1, np.float32).reshape(np.asarray(in1).shape[0], -1)
        return (np.maximum((a0 - np.square(a1) * np.asarray(s1, np.float32)) * imm2, 0.0)
                + np.asarray(s0, np.float32)).astype(np.float32)

    def hb_ref(in0, in1, s0, s1, imm2):
        a0 = np.asarray(in0, np.float32).reshape(np.asarray(in0).shape[0], -1)
        return ((a0 - np.asarray(s0, np.float32))
                * np.asarray(s1, np.float32)).astype(np.float32)

    t = Src0 + Src1
    specs = [
        ("GAT_SCORE_ANT", Spec(body=maxx(t, t * C2), reference=score_ref), "score"),
        ("GAT_VAR_ANT", Spec(body=relu((Src0 - sq(Src1) * C1) * C2) + C0, reference=var_ref), "var"),
        ("GAT_HB_ANT", Spec(body=(Src0 - C0) * C1, reference=hb_ref), "hb"),
    ]
    for name, spec, key in specs:
        op = dvo.DveOp(name, spec, subdim=False, uops_sha={})
        opc = max(dvo._SUB_OPCODE_FOR_NAME.values()) + 1
        assert opc < 0x20, "custom DVE opcode table full"
        dvo.OPS.append(op)
        dvo._SUB_OPCODE_FOR_NAME[name] = opc
        dvo.CUSTOM_DVE_SPECS[name] = op.spec
        for ver in ("v3",):
            try:
                op.compile(ver)
            except ValueError as e:
                m = re.search(ver + r": ([0-9a-f]+)", str(e))
                if not m:
                    raise
                op.uops_sha[ver] = m.group(1)
            op.compile(ver)
        _OPS[key] = op


# --------------------------------------------------------------------------
# Host-side graph partitioning (pure index work)
# --------------------------------------------------------------------------

def _padded_row(n):
    """Global node id -> row in the padded (8*5120) table layout."""
    return (n // SL) * SLP + (n % SL)


def _prep_plan(edge_index):
    src = edge_index[0].astype(np.int64)
    dst = edge_index[1].astype(np.int64)
    prow = _padded_row(src)

    # per (core, local block, half) edge lists
    seg = {}
    for c in range(NCORES):
        m = (dst >= c * SL) & (dst < (c + 1) * SL)
        sp = prow[m]
        dl = dst[m] - c * SL
        order = np.argsort(dl, kind="stable")
        sp = sp[order]
        dl = dl[order]
        blk = dl // 128
        lo = sp < LOROWS
        for b in range(NB):
            mb = blk == b
            for half, mh in (("lo", mb & lo), ("hi", mb & ~lo)):
                rows = sp[mh] - (0 if half == "lo" else LOROWS)
                seg[(c, b, half)] = (rows.astype(np.int64),
                                     (dl[mh] - b * 128).astype(np.int64),
                                     sp[mh].astype(np.int64))

    # common tile structure: per (block, half) max tile count across cores
    ntile = {}
    for b in range(NB):
        for half in ("lo", "hi"):
            mx = max(len(seg[(c, b, half)][0]) for c in range(NCORES))
            ntile[(b, half)] = max(1, -(-mx // 128))

    # global tile order: groups of GRP blocks; within a group all lo tiles
    # (block-major) then all hi tiles
    tiles = []
    lo_tile_of = {}
    hi_tile_of = {}
    nlo = nhi = 0
    groups = []
    for g0 in range(0, NB, GRP):
        blocks = list(range(g0, min(g0 + GRP, NB)))
        g = {"blocks": blocks, "lo0": nlo, "hi0": nhi, "tiles": []}
        for b in blocks:
            lo_tile_of[b] = nlo
            nlo += ntile[(b, "lo")]
            hi_tile_of[b] = nhi
            nhi += ntile[(b, "hi")]
        for b in blocks:
            for k in range(ntile[(b, "lo")]):
                g["tiles"].append((b, "lo", lo_tile_of[b] + k))
                tiles.append((b, "lo", lo_tile_of[b] + k))
            for k in range(ntile[(b, "hi")]):
                g["tiles"].append((b, "hi", hi_tile_of[b] + k))
                tiles.append((b, "hi", hi_tile_of[b] + k))
        g["nlo"] = nlo - g["lo0"]
        g["nhi"] = nhi - g["hi0"]
        groups.append(g)

    ntot = len(tiles)

    first = {}
    last = {}
    for ti, (b, half, _) in enumerate(tiles):
        if b not in first:
            first[b] = ti
        last[b] = ti

    # per-core arrays
    idx_lo = np.zeros((NCORES, 128, nlo * 8), np.int16)
    idx_hi = np.zeros((NCORES, 128, nhi * 8), np.int16)
    dstloc = np.full((NCORES, 128, ntot), 999.0, np.float32)
    srcg = np.full((NCORES, 128, ntot), -1, np.int64)    # global padded src row
    dstg = np.full((NCORES, 128, ntot), -1, np.int64)    # global padded dst row
    for c in range(NCORES):
        for b in range(NB):
            for half, base_of, arr in (("lo", lo_tile_of, idx_lo), ("hi", hi_tile_of, idx_hi)):
                rows, dl, spg = seg[(c, b, half)]
                nt = ntile[(b, half)]
                rpad = np.zeros(nt * 128, np.int64)
                rpad[: len(rows)] = rows
                wrapped = rpad.reshape(nt * 8, 16).T  # idx i -> [i%16, i//16]
                arr[c, :, base_of[b] * 8: base_of[b] * 8 + nt * 8] = np.tile(
                    wrapped.astype(np.int16), (8, 1))
        for b in range(NB):
            for half, base_of in (("lo", lo_tile_of), ("hi", hi_tile_of)):
                rows, dl, spg = seg[(c, b, half)]
                nt = ntile[(b, half)]
                dpad = np.full(nt * 128, 999.0, np.float32)
                dpad[: len(dl)] = dl.astype(np.float32)
                span = np.full(nt * 128, -1, np.int64)
                span[: len(spg)] = spg
                dgp = np.full(nt * 128, -1, np.int64)
                dgp[: len(dl)] = c * SLP + b * 128 + dl
                tis = [ti for ti, (bb, hh, _) in enumerate(tiles) if bb == b and hh == half]
                for k, ti in enumerate(tis):
                    dstloc[c, :, ti] = dpad[k * 128: (k + 1) * 128]
                    srcg[c, :, ti] = span[k * 128: (k + 1) * 128]
                    dstg[c, :, ti] = dgp[k * 128: (k + 1) * 128]

    return {
        "groups": groups, "tiles": tiles, "first": first, "last": last,
        "nlo": nlo, "nhi": nhi, "ntot": ntot,
        "idx_lo": idx_lo, "idx_hi": idx_hi, "dstloc": dstloc,
        "srcg": srcg, "dstg": dstg,
    }


def _prep_pool(batch):
    """Per-core one-hot graph-membership tiles [128, NB*G] bf16 (0 for pad)."""
    ghot = np.zeros((NCORES, 128, NB * G), NPBF)
    for c in range(NCORES):
        bslice = batch[c * SL: (c + 1) * SL].astype(np.int64)
        oh = np.zeros((SLP, G), np.float32)
        oh[np.arange(SL), bslice] = 1.0
        ghot[c] = oh.reshape(NB, 128, G).transpose(1, 0, 2).reshape(128, NB * G).astype(NPBF)
    return ghot


def _edge_cols(plan, s_full, d_full):
    """Host gather of per-edge score columns (pad slots -> 0)."""
    sg, dg = plan["srcg"], plan["dstg"]
    scol = np.where(sg >= 0, s_full[np.maximum(sg, 0)], 0.0).astype(np.float32)
    ccol = np.where(dg >= 0, d_full[np.maximum(dg, 0)], 0.0).astype(np.float32)
    return scol, ccol


# --------------------------------------------------------------------------
# Program builders
# --------------------------------------------------------------------------

def _new_nc():
    return bacc.Bacc("TRN2", target_bir_lowering=False, debug=False,
                     enable_asserts=False, num_devices=NCORES)


def _build_p0():
    """Per-core T1 slice (rows [h1|1]) + s1/d1 node scalars."""
    nc = _new_nc()
    xt_in = nc.dram_tensor("xT", [F1, SLP], BF16, kind="ExternalInput").ap()
    wext_in = nc.dram_tensor("w1ext", [F1, F1 + 2], BF16, kind="ExternalInput").ap()
    t1_out = nc.dram_tensor("t1slice", [SLP, ROW1], BF16, kind="ExternalOutput").ap()
    sd_out = nc.dram_tensor("sd1", [SLP, 2], F32, kind="ExternalOutput").ap()

    with tile.TileContext(nc, num_cores=NCORES) as tc, ExitStack() as ctx:
        singles = ctx.enter_context(tc.tile_pool(name="singles", bufs=1))
        ps = ctx.enter_context(tc.tile_pool(name="ps", bufs=4, space="PSUM"))

        xt = singles.tile([128, SLP], BF16)
        nc.sync.dma_start(xt[:, 0:SLP // 2], xt_in[:, 0:SLP // 2])
        nc.sync.dma_start(xt[:, SLP // 2:], xt_in[:, SLP // 2:])
        wext = singles.tile([128, F1 + 2], BF16)
        nc.sync.dma_start(wext, wext_in)

        t1stage = singles.tile([128, NB, ROW1], BF16)
        sdstage = singles.tile([128, NB, 2], F32)

        for b0 in range(0, NB, 2):
            hps = ps.tile([128, 2, F1 + 2], F32, tag="hps", name=f"hps{b0}")
            for j in range(2):
                b = b0 + j
                nc.tensor.matmul(hps[:, j, :], xt[:, b * 128:(b + 1) * 128], wext,
                                 start=True, stop=True)
            nc.scalar.activation(t1stage[:, b0:b0 + 2, 0:F1], hps[:, :, 0:F1],
                                 mybir.ActivationFunctionType.Copy)
            nc.vector.tensor_copy(sdstage[:, b0:b0 + 2, :], hps[:, :, F1:F1 + 2])

        t1ap = t1_out.rearrange("(b p) c -> p b c", p=128)
        sdap = sd_out.rearrange("(b p) c -> p b c", p=128)
        nc.sync.dma_start(t1ap[:, 0:NB // 2, :], t1stage[:, 0:NB // 2, :])
        nc.sync.dma_start(sdap[:, 0:NB // 2, :], sdstage[:, 0:NB // 2, :])
        nc.sync.dma_start(t1ap[:, NB // 2:, :], t1stage[:, NB // 2:, :])
        nc.sync.dma_start(sdap[:, NB // 2:, :], sdstage[:, NB // 2:, :])
    nc.finalize()
    return nc


def _build_msg_layer(plan, layer, fast):
    """P1 (layer=1) / P2 (layer=2): gather + attention + scatter + post."""
    F = F1 if layer == 1 else F2
    ROW = ROW1 if layer == 1 else ROW2
    nc = _new_nc()

    tlo_in = nc.dram_tensor("tlo", [LOROWS, ROW], BF16, kind="ExternalInput").ap()
    thi_in = nc.dram_tensor("thi", [LOROWS, ROW], BF16, kind="ExternalInput").ap()
    ilo_in = nc.dram_tensor("idxlo", [128, plan["nlo"] * 8], I16, kind="ExternalInput").ap()
    ihi_in = nc.dram_tensor("idxhi", [128, plan["nhi"] * 8], I16, kind="ExternalInput").ap()
    ntot = plan["ntot"]
    dl_in = nc.dram_tensor("dstloc", [128, ntot], F32, kind="ExternalInput").ap()
    ss_in = nc.dram_tensor("scol", [128, ntot], F32, kind="ExternalInput").ap()
    cc_in = nc.dram_tensor("ccol", [128, ntot], F32, kind="ExternalInput").ap()
    town_in = nc.dram_tensor("town", [SLP, ROW], BF16, kind="ExternalInput").ap()
    sself_in = nc.dram_tensor("sself", [128, NB], F32, kind="ExternalInput").ap()
    cself_in = nc.dram_tensor("cself", [128, NB], F32, kind="ExternalInput").ap()
    if not fast:
        b_in = nc.dram_tensor("bias", [F], F32, kind="ExternalInput").ap()
        g_in = nc.dram_tensor("gamma", [F], F32, kind="ExternalInput").ap()
        be_in = nc.dram_tensor("beta", [F], F32, kind="ExternalInput").ap()
    if layer == 1:
        w2_in = nc.dram_tensor("w2ext", [F2 * 2, F2 + 2], BF16, kind="ExternalInput").ap()
        id_in = nc.dram_tensor("identb", [128, 128], BF16, kind="ExternalInput").ap()
        t2_out = nc.dram_tensor("t2slice", [SLP, ROW2], BF16, kind="ExternalOutput").ap()
        sd2_out = nc.dram_tensor("sd2", [SLP, 2], F32, kind="ExternalOutput").ap()
    else:
        gh_in = nc.dram_tensor("ghot", [128, NB * G], BF16, kind="ExternalInput").ap()
        pool_out = nc.dram_tensor("pooled", [G, F2 + 1], F32, kind="ExternalOutput").ap()

    groups, tiles = plan["groups"], plan["tiles"]
    first, last = plan["first"], plan["last"]
    score_op, var_op, hb_op = _OPS["score"], _OPS["var"], _OPS["hb"]

    with tile.TileContext(nc, num_cores=NCORES) as tc, ExitStack() as ctx:
        singles = ctx.enter_context(tc.tile_pool(name="singles", bufs=1))
        gsb = ctx.enter_context(tc.tile_pool(name="gsb", bufs=3))
        msb = ctx.enter_context(tc.tile_pool(name="msb", bufs=4))
        posb = ctx.enter_context(tc.tile_pool(name="posb", bufs=3))
        agg_ps = ctx.enter_context(tc.tile_pool(name="aggps", bufs=3, space="PSUM"))
        aggz_ps = (ctx.enter_context(tc.tile_pool(name="aggzps", bufs=3, space="PSUM"))
                   if layer == 1 else None)
        aux_ps = (ctx.enter_context(tc.tile_pool(name="auxps", bufs=2, space="PSUM"))
                  if (layer == 1 or not fast) else None)
        pps = ctx.enter_context(tc.tile_pool(name="pps", bufs=1, space="PSUM")) if layer == 2 else None

        ones_col = singles.tile([128, 1], BF16)
        nc.vector.memset(ones_col, 1.0)
        iota_b = singles.tile([128, 128], BF16)
        nc.gpsimd.iota(iota_b, [[1, 128]], base=0, channel_multiplier=0,
                       allow_small_or_imprecise_dtypes=True)

        ilo_sb = singles.tile([128, plan["nlo"] * 8], I16)
        ihi_sb = singles.tile([128, plan["nhi"] * 8], I16)
        for g in plan["groups"]:
            if g["nlo"]:
                nc.sync.dma_start(ilo_sb[:, g["lo0"] * 8:(g["lo0"] + g["nlo"]) * 8],
                                  ilo_in[:, g["lo0"] * 8:(g["lo0"] + g["nlo"]) * 8])
            if g["nhi"]:
                nc.sync.dma_start(ihi_sb[:, g["hi0"] * 8:(g["hi0"] + g["nhi"]) * 8],
                                  ihi_in[:, g["hi0"] * 8:(g["hi0"] + g["nhi"]) * 8])
        dl_sb = singles.tile([128, ntot], F32)
        nc.sync.dma_start(dl_sb, dl_in)
        ss_sb = singles.tile([128, ntot], F32)
        nc.sync.dma_start(ss_sb, ss_in)
        cc_sb = singles.tile([128, ntot], F32)
        nc.sync.dma_start(cc_sb, cc_in)

        town_sb = singles.tile([128, NB, ROW], BF16)
        town_ap = town_in.rearrange("(b p) c -> p b c", p=128)
        for g0 in range(0, NB, GRP):
            g1 = min(g0 + GRP, NB)
            nc.sync.dma_start(town_sb[:, g0:g1, :], town_ap[:, g0:g1, :])
        sself_sb = singles.tile([128, NB], F32)
        nc.sync.dma_start(sself_sb, sself_in)
        cself_sb = singles.tile([128, NB], F32)
        nc.sync.dma_start(cself_sb, cself_in)
        iota_col = singles.tile([128, 1], F32)
        nc.gpsimd.iota(iota_col, [[0, 1]], base=0, channel_multiplier=1,
                       allow_small_or_imprecise_dtypes=True)

        # ex = exp(lrelu(s+c)) for the whole phase
        sc = singles.tile([128, ntot], F32)
        nc.vector._custom_dve(score_op, out=sc, in0=ss_sb, in1=cc_sb, imm2=0.2)
        ex = singles.tile([128, ntot], F32)
        nc.scalar.activation(ex, sc, mybir.ActivationFunctionType.Exp)
        scs = singles.tile([128, NB], F32)
        nc.vector._custom_dve(score_op, out=scs, in0=sself_sb, in1=cself_sb, imm2=0.2)
        exs = singles.tile([128, NB], F32)
        nc.scalar.activation(exs, scs, mybir.ActivationFunctionType.Exp)

        if not fast:
            ones_row = singles.tile([1, 128], F32)
            nc.vector.memset(ones_row, 1.0)

            def bcast_row(dram_row_ap, width, nm):
                t = singles.tile([1, width], F32, tag="bcrow", name=f"bcrow_{nm}")
                nc.sync.dma_start(t, dram_row_ap)
                p = aux_ps.tile([128, width], F32, tag="aux", name=f"bcps_{nm}")
                nc.tensor.matmul(p, ones_row, t[0:1, 0:width], start=True, stop=True)
                out = singles.tile([128, width], F32, name=f"bcast_{nm}")
                nc.scalar.activation(out, p, mybir.ActivationFunctionType.Copy)
                return out

            bB = bcast_row(b_in.rearrange("(a b) -> a b", a=1), F, "b")
            gB = bcast_row(g_in.rearrange("(a b) -> a b", a=1), F, "g")
            beB = bcast_row(be_in.rearrange("(a b) -> a b", a=1), F, "be")

        if layer == 1:
            w2ext = singles.tile([F2 * 2, F2 + 2], BF16)
            nc.sync.dma_start(w2ext, w2_in)
            identb = singles.tile([128, 128], BF16)
            nc.sync.dma_start(identb, id_in)
            t2stage = singles.tile([128, NB, ROW2], BF16)
            nc.gpsimd.memset(t2stage, 0.0)
            nc.gpsimd.memset(t2stage[:, :, F2:F2 + 1], 1.0)
            sd2stage = singles.tile([128, NB, 2], F32)
        else:
            gh_sb = singles.tile([128, NB * G], BF16)
            nc.sync.dma_start(gh_sb, gh_in)
            pool_psum = pps.tile([G, F2 + 1], F32)

        def postproc(b, agg, aggz):
            zsrc = aggz if layer == 1 else agg[:, F:F + 1]
            rz = posb.tile([128, 1], F32, tag="rz")
            nc.vector.reciprocal_approx_fast(rz, zsrc)
            r = posb.tile([128, F], F32, tag="r")
            msum = posb.tile([128, 1], F32, tag="msum")
            if fast:
                nc.scalar.activation(r, agg[:, 0:F], mybir.ActivationFunctionType.Relu,
                                     scale=rz, accum_out=msum)
            else:
                u = posb.tile([128, F], F32, tag="u")
                nc.vector.scalar_tensor_tensor(u, bB, zsrc, agg[:, 0:F],
                                               mybir.AluOpType.mult, mybir.AluOpType.add)
                nc.scalar.activation(r, u, mybir.ActivationFunctionType.Relu,
                                     scale=rz, accum_out=msum)
            scr = posb.tile([128, F], F32, tag="scr")
            ssq = posb.tile([128, 1], F32, tag="ssq")
            nc.scalar.activation(scr, r, mybir.ActivationFunctionType.Square,
                                 accum_out=ssq)
            vpe = posb.tile([128, 1], F32, tag="vpe")
            nc.vector._custom_dve(var_op, out=vpe, in0=ssq, in1=msum,
                                  s0=EPS, s1=1.0 / F, imm2=1.0 / F)
            mu = posb.tile([128, 1], F32, tag="mu")
            nc.vector.tensor_scalar(mu, msum, 1.0 / F, None, mybir.AluOpType.mult)
            sd = posb.tile([128, 1], F32, tag="sd")
            nc.scalar.activation(sd, vpe, mybir.ActivationFunctionType.Sqrt)
            rsd = posb.tile([128, 1], F32, tag="rsd")
            nc.vector.reciprocal_approx_fast(rsd, sd)

            if layer == 1:
                hb = posb.tile([128, F], BF16, tag="hb")
            else:
                hb = posb.tile([128, F2 + 1], BF16, tag="hb")
                nc.vector.memset(hb[:, F2:F2 + 1], 1.0)
            if fast:
                nc.vector._custom_dve(hb_op, out=hb[:, 0:F], in0=r, s0=mu, s1=rsd)
            else:
                h0 = posb.tile([128, F], F32, tag="h0")
                nc.vector._custom_dve(hb_op, out=h0, in0=r, s0=mu, s1=rsd)
                h1 = posb.tile([128, F], F32, tag="h1")
                nc.vector.tensor_tensor(h1, h0, gB, mybir.AluOpType.mult)
                nc.vector.tensor_tensor(hb[:, 0:F], h1, beB, mybir.AluOpType.add)

            if layer == 1:
                trp = aux_ps.tile([128, 128], BF16, tag="aux", name=f"trp{b}")
                nc.tensor.transpose(trp, hb, identb)
                lnbT = posb.tile([128, 128], BF16, tag="lnbT")
                nc.scalar.activation(lnbT, trp, mybir.ActivationFunctionType.Copy)
                proj = aux_ps.tile([128, F2 + 2], F32, tag="aux", name=f"proj{b}")
                nc.tensor.matmul(proj, lnbT, w2ext[0:128, :], start=True, stop=True)
                nc.scalar.activation(t2stage[:, b, 0:F2], proj[:, 0:F2],
                                     mybir.ActivationFunctionType.Copy)
                nc.vector.tensor_copy(sd2stage[:, b, :], proj[:, F2:F2 + 2])
            else:
                nc.tensor.matmul(pool_psum, gh_sb[:, b * G:(b + 1) * G], hb,
                                 start=(b == 0), stop=(b == NB - 1))

        agg_of = {}
        for g in groups:
            nlo_g, nhi_g = g["nlo"], g["nhi"]
            glo = gsb.tile([128, max(nlo_g, 1), ROW], BF16, tag="glo")
            ghi = gsb.tile([128, max(nhi_g, 1), ROW], BF16, tag="ghi")
            if nlo_g:
                nc.gpsimd.dma_gather(glo[:, 0:nlo_g, :], tlo_in,
                                     ilo_sb[:, g["lo0"] * 8:(g["lo0"] + nlo_g) * 8],
                                     nlo_g * 128, nlo_g * 128, ROW, single_packet=False)
            if nhi_g:
                nc.gpsimd.dma_gather(ghi[:, 0:nhi_g, :], thi_in,
                                     ihi_sb[:, g["hi0"] * 8:(g["hi0"] + nhi_g) * 8],
                                     nhi_g * 128, nhi_g * 128, ROW, single_packet=False)
            t0 = tiles.index(g["tiles"][0]) if g["tiles"] else 0

            for k, (b, half, spos) in enumerate(g["tiles"]):
                ti = t0 + k
                if half == "lo":
                    gt = glo[:, spos - g["lo0"], :]
                else:
                    gt = ghi[:, spos - g["hi0"], :]
                if b not in agg_of:
                    if layer == 1:
                        agg_of[b] = (agg_ps.tile([128, F], F32, tag="agg", name=f"agg{b}"),
                                     aggz_ps.tile([128, 1], F32, tag="aggz", name=f"aggz{b}"))
                    else:
                        agg_of[b] = (agg_ps.tile([128, F + 1], F32, tag="agg", name=f"agg{b}"),
                                     None)
                    ms = msb.tile([128, 128], BF16, tag="m", name=f"mself{b}")
                    nc.vector.tensor_scalar(ms, iota_b, iota_col, exs[:, b:b + 1],
                                            mybir.AluOpType.is_equal, mybir.AluOpType.mult)
                    at_, az_ = agg_of[b]
                    if layer == 1:
                        nc.tensor.matmul(at_, ms, town_sb[:, b, 0:F], start=True, stop=False)
                        nc.tensor.matmul(az_, ms, ones_col, start=True, stop=False)
                    else:
                        nc.tensor.matmul(at_, ms, town_sb[:, b, 0:F + 1], start=True, stop=False)
                m = msb.tile([128, 128], BF16, tag="m")
                nc.vector.tensor_scalar(m, iota_b, dl_sb[:, ti:ti + 1], ex[:, ti:ti + 1],
                                        mybir.AluOpType.is_equal, mybir.AluOpType.mult)
                aggt, aggzt = agg_of[b]
                if layer == 1:
                    nc.tensor.matmul(aggt, m, gt[:, 0:F],
                                     start=False, stop=(ti == last[b]))
                    nc.tensor.matmul(aggzt, m, ones_col,
                                     start=False, stop=(ti == last[b]))
                else:
                    nc.tensor.matmul(aggt, m, gt[:, 0:F + 1],
                                     start=False, stop=(ti == last[b]))
                if ti == last[b]:
                    a_, z_ = agg_of.pop(b)
                    postproc(b, a_, z_)
                    if layer == 1 and (b + 1) % 10 == 0 and b < NB - 1:
                        q0 = b + 1 - 10
                        nc.sync.dma_start(
                            t2_out.rearrange("(b p) c -> p b c", p=128)[:, q0:b + 1, :],
                            t2stage[:, q0:b + 1, :])
                        nc.sync.dma_start(
                            sd2_out.rearrange("(b p) c -> p b c", p=128)[:, q0:b + 1, :],
                            sd2stage[:, q0:b + 1, :])

        if layer == 1:
            t2ap = t2_out.rearrange("(b p) c -> p b c", p=128)
            sd2ap = sd2_out.rearrange("(b p) c -> p b c", p=128)
            nc.sync.dma_start(t2ap[:, NB - 10:, :], t2stage[:, NB - 10:, :])
            nc.sync.dma_start(sd2ap[:, NB - 10:, :], sd2stage[:, NB - 10:, :])
        else:
            pout = singles.tile([G, F2 + 1], F32)
            nc.vector.tensor_copy(pout, pool_psum)
            nc.sync.dma_start(pool_out, pout)
    nc.finalize()
    return nc


def _build_p3():
    nc = _new_nc()
    pin = nc.dram_tensor("pall", [G, NCORES * (F2 + 1)], F32, kind="ExternalInput").ap()
    wl_in = nc.dram_tensor("Wl", [F2, F2], F32, kind="ExternalInput").ap()
    bl_in = nc.dram_tensor("bl", [F2], F32, kind="ExternalInput").ap()
    wc_in = nc.dram_tensor("Wc", [1, F2], F32, kind="ExternalInput").ap()
    bc_in = nc.dram_tensor("bc", [1], F32, kind="ExternalInput").ap()
    id_in = nc.dram_tensor("ident", [128, 128], F32, kind="ExternalInput").ap()
    out = nc.dram_tensor("out", [G], F32, kind="ExternalOutput").ap()

    with tile.TileContext(nc, num_cores=NCORES) as tc, ExitStack() as ctx:
        singles = ctx.enter_context(tc.tile_pool(name="singles", bufs=1))
        ps = ctx.enter_context(tc.tile_pool(name="ps", bufs=4, space="PSUM"))

        ident = singles.tile([128, 128], F32)
        nc.sync.dma_start(ident, id_in)
        acc = singles.tile([G, (F2 + 1) * NCORES], F32)
        nc.sync.dma_start(acc, pin)
        tots = [singles.tile([G, F2 + 1], F32, tag=f"tot{i}", name=f"tot{i}") for i in range(NCORES - 1)]
        nc.vector.tensor_tensor(tots[0], acc[:, 0:F2 + 1], acc[:, F2 + 1:2 * (F2 + 1)],
                                mybir.AluOpType.add)
        for c in range(2, NCORES):
            nc.vector.tensor_tensor(tots[c - 1], tots[c - 2],
                                    acc[:, c * (F2 + 1):(c + 1) * (F2 + 1)],
                                    mybir.AluOpType.add)
        tot = tots[NCORES - 2]
        cnt = singles.tile([G, 1], F32)
        nc.vector.tensor_scalar(cnt, tot[:, F2:F2 + 1], 1.0, None, mybir.AluOpType.max)
        rc = singles.tile([G, 1], F32)
        nc.vector.reciprocal(rc, cnt)
        pm = singles.tile([G, F2], F32)
        nc.vector.tensor_scalar(pm, tot[:, 0:F2], rc, None, mybir.AluOpType.mult)
        pmT_ps = ps.tile([F2, G], F32, tag="ps")
        nc.tensor.transpose(pmT_ps, pm, ident[0:G, 0:G])
        pmT = singles.tile([F2, G], F32)
        nc.vector.tensor_copy(pmT, pmT_ps)

        wl_sb = singles.tile([F2, F2], F32)
        nc.sync.dma_start(wl_sb, wl_in)
        wlt_ps = ps.tile([F2, F2], F32, tag="ps")
        nc.tensor.transpose(wlt_ps, wl_sb, ident[0:F2, 0:F2])
        wlt = singles.tile([F2, F2], F32)
        nc.vector.tensor_copy(wlt, wlt_ps)
        bl_sb = singles.tile([F2, 1], F32)
        nc.sync.dma_start(bl_sb, bl_in.rearrange("(a b) -> a b", b=1))
        y1_ps = ps.tile([F2, G], F32, tag="ps")
        nc.tensor.matmul(y1_ps, wlt, pmT, start=True, stop=True)
        y1 = singles.tile([F2, G], F32)
        nc.scalar.activation(y1, y1_ps, mybir.ActivationFunctionType.Identity, bias=bl_sb)
        wc_sb = singles.tile([F2, 1], F32)
        nc.sync.dma_start(wc_sb, wc_in.rearrange("a b -> b a"))
        bc_sb = singles.tile([1, 1], F32)
        nc.sync.dma_start(bc_sb, bc_in.rearrange("(a b) -> a b", b=1))
        y2_ps = ps.tile([1, G], F32, tag="ps")
        nc.tensor.matmul(y2_ps, wc_sb, y1, start=True, stop=True)
        y2 = singles.tile([1, G], F32)
        nc.scalar.activation(y2, y2_ps, mybir.ActivationFunctionType.Identity, bias=bc_sb)
        nc.sync.dma_start(out.rearrange("(a b) -> a b", a=1), y2)
    nc.finalize()
    return nc


# --------------------------------------------------------------------------
# Entry point
# --------------------------------------------------------------------------

def _note(rr, name):
    global LAST_EXEC_NS
    ns = rr.exec_time_ns
    if ns is not None:
        EXEC_NS.append((name, ns, None))
        LAST_EXEC_NS += ns


def kernel(x, edge_index, batch, W1, a1_src, a1_dst, b1, g1, be1,
           W2, a2_src, a2_dst, b2, g2, be2, Wl, bl, Wc, bc):
    _register_ops()
    x = np.asarray(x, np.float32)
    edge_index = np.asarray(edge_index)
    batch = np.asarray(batch)
    W1 = np.asarray(W1, np.float32)
    W2 = np.asarray(W2, np.float32)
    b1 = np.asarray(b1, np.float32)
    g1 = np.asarray(g1, np.float32)
    be1 = np.asarray(be1, np.float32)
    b2 = np.asarray(b2, np.float32)
    g2 = np.asarray(g2, np.float32)
    be2 = np.asarray(be2, np.float32)
    ident = np.eye(128, dtype=np.float32)
    identb = np.eye(128, dtype=NPBF)

    fast1 = (not b1.any()) and np.all(g1 == 1.0) and (not be1.any())
    fast2 = (not b2.any()) and np.all(g2 == 1.0) and (not be2.any())

    plan = _prep_plan(edge_index)
    ghot = _prep_pool(batch)

    # host-side weight prep
    w1ext = np.concatenate([W1.T,
                            (W1.T @ np.asarray(a1_src, np.float32))[:, None],
                            (W1.T @ np.asarray(a1_dst, np.float32))[:, None]],
                           axis=1).astype(NPBF)                     # [128, 130]
    w2ext = np.concatenate([W2.T,
                            (W2.T @ np.asarray(a2_src, np.float32))[:, None],
                            (W2.T @ np.asarray(a2_dst, np.float32))[:, None]],
                           axis=1).astype(NPBF)                     # [128, 66]

    # ---- P0: table build -------------------------------------------------
    xT = np.zeros((NCORES, F1, SLP), NPBF)
    for c in range(NCORES):
        xT[c, :, :SL] = x[c * SL:(c + 1) * SL].T.astype(NPBF)
    nc0 = _build_p0()
    in0 = [{"xT": xT[c], "w1ext": w1ext} for c in range(NCORES)]
    _rr = _RUN(nc0, in0, core_ids=list(range(NCORES)), trace=TRACE)
    _note(_rr, "P0")
    r0 = _rr.results
    t1_full = np.concatenate([r0[c]["t1slice"] for c in range(NCORES)], axis=0)
    sd1_full = np.concatenate([r0[c]["sd1"] for c in range(NCORES)], axis=0)
    scol1, ccol1 = _edge_cols(plan, sd1_full[:, 0], sd1_full[:, 1])

    # ---- P1: layer 1 -----------------------------------------------------
    nc1 = _build_msg_layer(plan, 1, fast1)
    in1 = []
    for c in range(NCORES):
        d = {"tlo": t1_full[:LOROWS], "thi": t1_full[LOROWS:],
             "idxlo": plan["idx_lo"][c], "idxhi": plan["idx_hi"][c],
             "dstloc": plan["dstloc"][c], "scol": scol1[c], "ccol": ccol1[c],
             "town": t1_full[c * SLP:(c + 1) * SLP],
             "sself": sd1_full[c * SLP:(c + 1) * SLP, 0].reshape(NB, 128).T.copy(),
             "cself": sd1_full[c * SLP:(c + 1) * SLP, 1].reshape(NB, 128).T.copy(),
             "w2ext": w2ext, "identb": identb}
        if not fast1:
            d.update({"bias": b1, "gamma": g1, "beta": be1})
        in1.append(d)
    _rr = _RUN(nc1, in1, core_ids=list(range(NCORES)), trace=TRACE)
    _note(_rr, "P1")
    r1 = _rr.results
    t2_full = np.concatenate([r1[c]["t2slice"] for c in range(NCORES)], axis=0)
    sd2_full = np.concatenate([r1[c]["sd2"] for c in range(NCORES)], axis=0)
    scol2, ccol2 = _edge_cols(plan, sd2_full[:, 0], sd2_full[:, 1])

    # ---- P2: layer 2 + partial pool -------------------------------------
    nc2 = _build_msg_layer(plan, 2, fast2)
    in2 = []
    for c in range(NCORES):
        d = {"tlo": t2_full[:LOROWS], "thi": t2_full[LOROWS:],
             "idxlo": plan["idx_lo"][c], "idxhi": plan["idx_hi"][c],
             "dstloc": plan["dstloc"][c], "scol": scol2[c], "ccol": ccol2[c],
             "town": t2_full[c * SLP:(c + 1) * SLP],
             "sself": sd2_full[c * SLP:(c + 1) * SLP, 0].reshape(NB, 128).T.copy(),
             "cself": sd2_full[c * SLP:(c + 1) * SLP, 1].reshape(NB, 128).T.copy(),
             "ghot": ghot[c]}
        if not fast2:
            d.update({"bias": b2, "gamma": g2, "beta": be2})
        in2.append(d)
    _rr = _RUN(nc2, in2, core_ids=list(range(NCORES)), trace=TRACE)
    _note(_rr, "P2")
    r2 = _rr.results
    pall = np.stack([r2[c]["pooled"] for c in range(NCORES)], axis=0)
    pall = np.ascontiguousarray(pall.transpose(1, 0, 2).reshape(G, NCORES * (F2 + 1)))

    # ---- P3: combine + MLP ----------------------------------------------
    nc3 = _build_p3()
    in3 = [{"pall": pall, "Wl": np.asarray(Wl, np.float32),
            "bl": np.asarray(bl, np.float32), "Wc": np.asarray(Wc, np.float32),
            "bc": np.asarray(bc, np.float32), "ident": ident} for c in range(NCORES)]
    _rr = _RUN(nc3, in3, core_ids=list(range(NCORES)), trace=TRACE)
    _note(_rr, "P3")
    r3 = _rr.results
    return np.asarray(r3[0]["out"], np.float32)


# revision 17
# speedup vs baseline: 1.0429x; 1.0429x over previous
"""Trainium2 Bass kernel for nn_BinGATConv (2-layer GAT + LN + mean-pool + MLP).

Strategy (8 NeuronCores, SPMD), v2:
  - Nodes dst-sharded: core c owns dst nodes [c*5000, (c+1)*5000); edges are
    1D-partitioned by dst on the host and sorted by (dst_block, src_half).
  - 4 sequential SPMD launches; the host only reshards/concats/gathers
    index-addressed STAGED VALUES between them (halo exchange):
      P0: per-core slice of the L1 gather table T1[n] = [h1(n)|1] (bf16)
          plus per-node attention scalars s1(n), d1(n) (f32).
      P1: L1 message passing + ReLU/LN + W2 projection -> T2 slice + s2/d2
      P2: L2 message passing + ReLU/LN + per-graph partial mean-pool
      P3: combine 8 partial pools + tiny MLP head (replicated)
  - Between launches the host gathers per-edge score columns
    scol[e] = s(src_e), ccol[e] = d(dst_e)  (pure indexed data movement).
  - On device per 128-edge tile:
      ex   = exp(lrelu(scol+ccol))            (one wide DVE op + one ACT exp
                                               for the entire phase)
      M    = (iota == dstloc_e) * ex_e        (one DVE tensor_scalar, bf16 4x)
      PSUM[d, 0:F+1] += M^T @ [h[src_e] | 1]  (one PE matmul per tile)
  - Per-dst-block postproc: softmax divide + ReLU + LayerNorm with the
    rsqrt computed as exp(-0.5*ln(var+eps)) so every ACT function of the
    phase lives in ONE activation table (no LoadActFuncSet thrash).
"""

import re
from contextlib import ExitStack

import ml_dtypes
import numpy as np

import concourse.bass as bass
import concourse.bacc as bacc
import concourse.mybir as mybir
import concourse.tile as tile
import concourse.dve_ops as dvo
from concourse.dve_spec import Spec, Src0, Src1, C0, C1, C2, eq, maxx, select, Idx, relu, sq
from concourse.bass_utils import run_bass_kernel_spmd

F32 = mybir.dt.float32
BF16 = mybir.dt.bfloat16
I16 = mybir.dt.int16
NPBF = ml_dtypes.bfloat16

NCORES = 8
N = 40000
E = 640000
G = 64
SL = N // NCORES          # 5000 nodes per core
SLP = 5120                # padded slice (40*128)
NB = SLP // 128           # 40 dst blocks per core
LOROWS = 4 * SLP          # 20480 rows in each table half
F1 = 128                  # layer-1 feature dim
F2 = 64                   # layer-2 feature dim
ROW1 = 128                # bf16 cols per T1 row (256B): [h1(128)]
ROW2 = 128                # bf16 cols per T2 row (256B): [h2(64) | 1 | pad]
GRP = 4                   # dst blocks per gather group
EPS = 1e-5

_OPS = {}
TRACE = False
LAST_EXEC_NS = 0
EXEC_NS = []
_RUN = run_bass_kernel_spmd


def _register_ops():
    if "GAT_SCORE_ANT" in dvo._SUB_OPCODE_FOR_NAME:
        for key, name in (("score", "GAT_SCORE_ANT"), ("var", "GAT_VAR_ANT"),
                          ("hb", "GAT_HB_ANT")):
            _OPS[key] = next(o for o in dvo.OPS if o.name == name)
        return

    def score_ref(in0, in1, s0, s1, imm2):
        a0 = np.asarray(in0, np.float32).reshape(np.asarray(in0).shape[0], -1)
        a1 = np.asarray(in1, np.float32).reshape(np.asarray(in1).shape[0], -1)
        t = a0 + a1
        return np.maximum(t, t * imm2).astype(np.float32)

    def var_ref(in0, in1, s0, s1, imm2):
        a0 = np.asarray(in0, np.float32).reshape(np.asarray(in0).shape[0], -1)
        a1 = np.asarray(in1, np.float32).reshape(np.asarray(in1).shape[0], -1)
        return (np.maximum((a0 - np.square(a1) * np.asarray(s1, np.float32)) * imm2, 0.0)
                + np.asarray(s0, np.float32)).astype(np.float32)

    def hb_ref(in0, in1, s0, s1, imm2):
        a0 = np.asarray(in0, np.float32).reshape(np.asarray(in0).shape[0], -1)
        return ((a0 - np.asarray(s0, np.float32))
                * np.asarray(s1, np.float32)).astype(np.float32)

    t = Src0 + Src1
    specs = [
        ("GAT_SCORE_ANT", Spec(body=maxx(t, t * C2), reference=score_ref), "score"),
        ("GAT_VAR_ANT", Spec(body=relu((Src0 - sq(Src1) * C1) * C2) + C0, reference=var_ref), "var"),
        ("GAT_HB_ANT", Spec(body=(Src0 - C0) * C1, reference=hb_ref), "hb"),
    ]
    for name, spec, key in specs:
        op = dvo.DveOp(name, spec, subdim=False, uops_sha={})
        opc = max(dvo._SUB_OPCODE_FOR_NAME.values()) + 1
        assert opc < 0x20, "custom DVE opcode table full"
        dvo.OPS.append(op)
        dvo._SUB_OPCODE_FOR_NAME[name] = opc
        dvo.CUSTOM_DVE_SPECS[name] = op.spec
        for ver in ("v3",):
            try:
                op.compile(ver)
            except ValueError as e:
                m = re.search(ver + r": ([0-9a-f]+)", str(e))
                if not m:
                    raise
                op.uops_sha[ver] = m.group(1)
            op.compile(ver)
        _OPS[key] = op


# --------------------------------------------------------------------------
# Host-side graph partitioning (pure index work)
# --------------------------------------------------------------------------

def _padded_row(n):
    """Global node id -> row in the padded (8*5120) table layout."""
    return (n // SL) * SLP + (n % SL)


def _prep_plan(edge_index):
    src = edge_index[0].astype(np.int64)
    dst = edge_index[1].astype(np.int64)
    prow = _padded_row(src)

    # per (core, local block, half) edge lists
    seg = {}
    for c in range(NCORES):
        m = (dst >= c * SL) & (dst < (c + 1) * SL)
        sp = prow[m]
        dl = dst[m] - c * SL
        order = np.argsort(dl, kind="stable")
        sp = sp[order]
        dl = dl[order]
        blk = dl // 128
        lo = sp < LOROWS
        for b in range(NB):
            mb = blk == b
            for half, mh in (("lo", mb & lo), ("hi", mb & ~lo)):
                rows = sp[mh] - (0 if half == "lo" else LOROWS)
                seg[(c, b, half)] = (rows.astype(np.int64),
                                     (dl[mh] - b * 128).astype(np.int64),
                                     sp[mh].astype(np.int64))

    # common tile structure: per (block, half) max tile count across cores
    ntile = {}
    for b in range(NB):
        for half in ("lo", "hi"):
            mx = max(len(seg[(c, b, half)][0]) for c in range(NCORES))
            ntile[(b, half)] = max(1, -(-mx // 128))

    # global tile order: groups of GRP blocks; within a group all lo tiles
    # (block-major) then all hi tiles
    tiles = []
    lo_tile_of = {}
    hi_tile_of = {}
    nlo = nhi = 0
    groups = []
    for g0 in range(0, NB, GRP):
        blocks = list(range(g0, min(g0 + GRP, NB)))
        g = {"blocks": blocks, "lo0": nlo, "hi0": nhi, "tiles": []}
        for b in blocks:
            lo_tile_of[b] = nlo
            nlo += ntile[(b, "lo")]
            hi_tile_of[b] = nhi
            nhi += ntile[(b, "hi")]
        for b in blocks:
            for k in range(ntile[(b, "lo")]):
                g["tiles"].append((b, "lo", lo_tile_of[b] + k))
                tiles.append((b, "lo", lo_tile_of[b] + k))
            for k in range(ntile[(b, "hi")]):
                g["tiles"].append((b, "hi", hi_tile_of[b] + k))
                tiles.append((b, "hi", hi_tile_of[b] + k))
        g["nlo"] = nlo - g["lo0"]
        g["nhi"] = nhi - g["hi0"]
        groups.append(g)

    ntot = len(tiles)

    first = {}
    last = {}
    for ti, (b, half, _) in enumerate(tiles):
        if b not in first:
            first[b] = ti
        last[b] = ti

    # per-core arrays
    idx_lo = np.zeros((NCORES, 128, nlo * 8), np.int16)
    idx_hi = np.zeros((NCORES, 128, nhi * 8), np.int16)
    dstloc = np.full((NCORES, 128, ntot), 999.0, np.float32)
    srcg = np.full((NCORES, 128, ntot), -1, np.int64)    # global padded src row
    dstg = np.full((NCORES, 128, ntot), -1, np.int64)    # global padded dst row
    for c in range(NCORES):
        for b in range(NB):
            for half, base_of, arr in (("lo", lo_tile_of, idx_lo), ("hi", hi_tile_of, idx_hi)):
                rows, dl, spg = seg[(c, b, half)]
                nt = ntile[(b, half)]
                rpad = np.zeros(nt * 128, np.int64)
                rpad[: len(rows)] = rows
                wrapped = rpad.reshape(nt * 8, 16).T  # idx i -> [i%16, i//16]
                arr[c, :, base_of[b] * 8: base_of[b] * 8 + nt * 8] = np.tile(
                    wrapped.astype(np.int16), (8, 1))
        for b in range(NB):
            for half, base_of in (("lo", lo_tile_of), ("hi", hi_tile_of)):
                rows, dl, spg = seg[(c, b, half)]
                nt = ntile[(b, half)]
                dpad = np.full(nt * 128, 999.0, np.float32)
                dpad[: len(dl)] = dl.astype(np.float32)
                span = np.full(nt * 128, -1, np.int64)
                span[: len(spg)] = spg
                dgp = np.full(nt * 128, -1, np.int64)
                dgp[: len(dl)] = c * SLP + b * 128 + dl
                tis = [ti for ti, (bb, hh, _) in enumerate(tiles) if bb == b and hh == half]
                for k, ti in enumerate(tis):
                    dstloc[c, :, ti] = dpad[k * 128: (k + 1) * 128]
                    srcg[c, :, ti] = span[k * 128: (k + 1) * 128]
                    dstg[c, :, ti] = dgp[k * 128: (k + 1) * 128]

    return {
        "groups": groups, "tiles": tiles, "first": first, "last": last,
        "nlo": nlo, "nhi": nhi, "ntot": ntot,
        "idx_lo": idx_lo, "idx_hi": idx_hi, "dstloc": dstloc,
        "srcg": srcg, "dstg": dstg,
    }


def _prep_pool(batch):
    """Per-core one-hot graph-membership tiles [128, NB*G] bf16 (0 for pad)."""
    ghot = np.zeros((NCORES, 128, NB * G), NPBF)
    for c in range(NCORES):
        bslice = batch[c * SL: (c + 1) * SL].astype(np.int64)
        oh = np.zeros((SLP, G), np.float32)
        oh[np.arange(SL), bslice] = 1.0
        ghot[c] = oh.reshape(NB, 128, G).transpose(1, 0, 2).reshape(128, NB * G).astype(NPBF)
    return ghot


def _edge_cols(plan, s_full, d_full):
    """Host gather of per-edge score columns (pad slots -> 0)."""
    sg, dg = plan["srcg"], plan["dstg"]
    scol = np.where(sg >= 0, s_full[np.maximum(sg, 0)], 0.0).astype(np.float32)
    ccol = np.where(dg >= 0, d_full[np.maximum(dg, 0)], 0.0).astype(np.float32)
    return scol, ccol


# --------------------------------------------------------------------------
# Program builders
# --------------------------------------------------------------------------

def _new_nc():
    return bacc.Bacc("TRN2", target_bir_lowering=False, debug=False,
                     enable_asserts=False, num_devices=NCORES)


def _build_p0():
    """Per-core T1 slice (rows [h1|1]) + s1/d1 node scalars."""
    nc = _new_nc()
    xt_in = nc.dram_tensor("xT", [F1, SLP], BF16, kind="ExternalInput").ap()
    wext_in = nc.dram_tensor("w1ext", [F1, F1 + 2], BF16, kind="ExternalInput").ap()
    t1_out = nc.dram_tensor("t1slice", [SLP, ROW1], BF16, kind="ExternalOutput").ap()
    sd_out = nc.dram_tensor("sd1", [SLP, 2], F32, kind="ExternalOutput").ap()

    with tile.TileContext(nc, num_cores=NCORES) as tc, ExitStack() as ctx:
        singles = ctx.enter_context(tc.tile_pool(name="singles", bufs=1))
        ps = ctx.enter_context(tc.tile_pool(name="ps", bufs=4, space="PSUM"))

        xt = singles.tile([128, SLP], BF16)
        nc.sync.dma_start(xt[:, 0:SLP // 2], xt_in[:, 0:SLP // 2])
        nc.sync.dma_start(xt[:, SLP // 2:], xt_in[:, SLP // 2:])
        wext = singles.tile([128, F1 + 2], BF16)
        nc.sync.dma_start(wext, wext_in)

        t1stage = singles.tile([128, NB, ROW1], BF16)
        sdstage = singles.tile([128, NB, 2], F32)

        for b0 in range(0, NB, 2):
            hps = ps.tile([128, 2, F1 + 2], F32, tag="hps", name=f"hps{b0}")
            for j in range(2):
                b = b0 + j
                nc.tensor.matmul(hps[:, j, :], xt[:, b * 128:(b + 1) * 128], wext,
                                 start=True, stop=True)
            nc.scalar.activation(t1stage[:, b0:b0 + 2, 0:F1], hps[:, :, 0:F1],
                                 mybir.ActivationFunctionType.Copy)
            nc.vector.tensor_copy(sdstage[:, b0:b0 + 2, :], hps[:, :, F1:F1 + 2])

        t1ap = t1_out.rearrange("(b p) c -> p b c", p=128)
        sdap = sd_out.rearrange("(b p) c -> p b c", p=128)
        nc.sync.dma_start(t1ap[:, 0:NB // 2, :], t1stage[:, 0:NB // 2, :])
        nc.sync.dma_start(sdap[:, 0:NB // 2, :], sdstage[:, 0:NB // 2, :])
        nc.sync.dma_start(t1ap[:, NB // 2:, :], t1stage[:, NB // 2:, :])
        nc.sync.dma_start(sdap[:, NB // 2:, :], sdstage[:, NB // 2:, :])
    nc.finalize()
    return nc


def _build_msg_layer(plan, layer, fast):
    """P1 (layer=1) / P2 (layer=2): gather + attention + scatter + post."""
    F = F1 if layer == 1 else F2
    ROW = ROW1 if layer == 1 else ROW2
    nc = _new_nc()

    tlo_in = nc.dram_tensor("tlo", [LOROWS, ROW], BF16, kind="ExternalInput").ap()
    thi_in = nc.dram_tensor("thi", [LOROWS, ROW], BF16, kind="ExternalInput").ap()
    ilo_in = nc.dram_tensor("idxlo", [128, plan["nlo"] * 8], I16, kind="ExternalInput").ap()
    ihi_in = nc.dram_tensor("idxhi", [128, plan["nhi"] * 8], I16, kind="ExternalInput").ap()
    ntot = plan["ntot"]
    dl_in = nc.dram_tensor("dstloc", [128, ntot], F32, kind="ExternalInput").ap()
    ss_in = nc.dram_tensor("scol", [128, ntot], F32, kind="ExternalInput").ap()
    cc_in = nc.dram_tensor("ccol", [128, ntot], F32, kind="ExternalInput").ap()
    town_in = nc.dram_tensor("town", [SLP, ROW], BF16, kind="ExternalInput").ap()
    sself_in = nc.dram_tensor("sself", [128, NB], F32, kind="ExternalInput").ap()
    cself_in = nc.dram_tensor("cself", [128, NB], F32, kind="ExternalInput").ap()
    if not fast:
        b_in = nc.dram_tensor("bias", [F], F32, kind="ExternalInput").ap()
        g_in = nc.dram_tensor("gamma", [F], F32, kind="ExternalInput").ap()
        be_in = nc.dram_tensor("beta", [F], F32, kind="ExternalInput").ap()
    if layer == 1:
        w2_in = nc.dram_tensor("w2ext", [F2 * 2, F2 + 2], BF16, kind="ExternalInput").ap()
        id_in = nc.dram_tensor("identb", [128, 128], BF16, kind="ExternalInput").ap()
        t2_out = nc.dram_tensor("t2slice", [SLP, ROW2], BF16, kind="ExternalOutput").ap()
        sd2_out = nc.dram_tensor("sd2", [SLP, 2], F32, kind="ExternalOutput").ap()
    else:
        gh_in = nc.dram_tensor("ghot", [128, NB * G], BF16, kind="ExternalInput").ap()
        pool_out = nc.dram_tensor("pooled", [G, F2 + 1], F32, kind="ExternalOutput").ap()

    groups, tiles = plan["groups"], plan["tiles"]
    first, last = plan["first"], plan["last"]
    score_op, var_op, hb_op = _OPS["score"], _OPS["var"], _OPS["hb"]

    with tile.TileContext(nc, num_cores=NCORES) as tc, ExitStack() as ctx:
        singles = ctx.enter_context(tc.tile_pool(name="singles", bufs=1))
        gsb = ctx.enter_context(tc.tile_pool(name="gsb", bufs=3))
        msb = ctx.enter_context(tc.tile_pool(name="msb", bufs=4))
        posb = ctx.enter_context(tc.tile_pool(name="posb", bufs=3))
        agg_ps = ctx.enter_context(tc.tile_pool(name="aggps", bufs=3, space="PSUM"))
        aggz_ps = (ctx.enter_context(tc.tile_pool(name="aggzps", bufs=3, space="PSUM"))
                   if layer == 1 else None)
        aux_ps = (ctx.enter_context(tc.tile_pool(name="auxps", bufs=2, space="PSUM"))
                  if (layer == 1 or not fast) else None)
        pps = ctx.enter_context(tc.tile_pool(name="pps", bufs=1, space="PSUM")) if layer == 2 else None

        ones_col = singles.tile([128, 1], BF16)
        nc.vector.memset(ones_col, 1.0)
        iota_b = singles.tile([128, 128], BF16)
        nc.gpsimd.iota(iota_b, [[1, 128]], base=0, channel_multiplier=0,
                       allow_small_or_imprecise_dtypes=True)

        ilo_sb = singles.tile([128, plan["nlo"] * 8], I16)
        nc.sync.dma_start(ilo_sb, ilo_in)
        ihi_sb = singles.tile([128, plan["nhi"] * 8], I16)
        nc.sync.dma_start(ihi_sb, ihi_in)
        dl_sb = singles.tile([128, ntot], F32)
        nc.sync.dma_start(dl_sb, dl_in)
        ss_sb = singles.tile([128, ntot], F32)
        nc.sync.dma_start(ss_sb, ss_in)
        cc_sb = singles.tile([128, ntot], F32)
        nc.sync.dma_start(cc_sb, cc_in)

        town_sb = singles.tile([128, NB, ROW], BF16)
        town_ap = town_in.rearrange("(b p) c -> p b c", p=128)
        for g0 in range(0, NB, GRP):
            g1 = min(g0 + GRP, NB)
            nc.sync.dma_start(town_sb[:, g0:g1, :], town_ap[:, g0:g1, :])
        sself_sb = singles.tile([128, NB], F32)
        nc.sync.dma_start(sself_sb, sself_in)
        cself_sb = singles.tile([128, NB], F32)
        nc.sync.dma_start(cself_sb, cself_in)
        iota_col = singles.tile([128, 1], F32)
        nc.gpsimd.iota(iota_col, [[0, 1]], base=0, channel_multiplier=1,
                       allow_small_or_imprecise_dtypes=True)

        # ex = exp(lrelu(s+c)) for the whole phase
        sc = singles.tile([128, ntot], F32)
        nc.vector._custom_dve(score_op, out=sc, in0=ss_sb, in1=cc_sb, imm2=0.2)
        ex = singles.tile([128, ntot], F32)
        nc.scalar.activation(ex, sc, mybir.ActivationFunctionType.Exp)
        scs = singles.tile([128, NB], F32)
        nc.vector._custom_dve(score_op, out=scs, in0=sself_sb, in1=cself_sb, imm2=0.2)
        exs = singles.tile([128, NB], F32)
        nc.scalar.activation(exs, scs, mybir.ActivationFunctionType.Exp)

        if not fast:
            ones_row = singles.tile([1, 128], F32)
            nc.vector.memset(ones_row, 1.0)

            def bcast_row(dram_row_ap, width, nm):
                t = singles.tile([1, width], F32, tag="bcrow", name=f"bcrow_{nm}")
                nc.sync.dma_start(t, dram_row_ap)
                p = aux_ps.tile([128, width], F32, tag="aux", name=f"bcps_{nm}")
                nc.tensor.matmul(p, ones_row, t[0:1, 0:width], start=True, stop=True)
                out = singles.tile([128, width], F32, name=f"bcast_{nm}")
                nc.scalar.activation(out, p, mybir.ActivationFunctionType.Copy)
                return out

            bB = bcast_row(b_in.rearrange("(a b) -> a b", a=1), F, "b")
            gB = bcast_row(g_in.rearrange("(a b) -> a b", a=1), F, "g")
            beB = bcast_row(be_in.rearrange("(a b) -> a b", a=1), F, "be")

        if layer == 1:
            w2ext = singles.tile([F2 * 2, F2 + 2], BF16)
            nc.sync.dma_start(w2ext, w2_in)
            identb = singles.tile([128, 128], BF16)
            nc.sync.dma_start(identb, id_in)
            t2stage = singles.tile([128, NB, ROW2], BF16)
            nc.gpsimd.memset(t2stage, 0.0)
            nc.gpsimd.memset(t2stage[:, :, F2:F2 + 1], 1.0)
            sd2stage = singles.tile([128, NB, 2], F32)
        else:
            gh_sb = singles.tile([128, NB * G], BF16)
            nc.sync.dma_start(gh_sb, gh_in)
            pool_psum = pps.tile([G, F2 + 1], F32)

        def postproc(b, agg, aggz):
            zsrc = aggz if layer == 1 else agg[:, F:F + 1]
            rz = posb.tile([128, 1], F32, tag="rz")
            nc.vector.reciprocal_approx_fast(rz, zsrc)
            r = posb.tile([128, F], F32, tag="r")
            msum = posb.tile([128, 1], F32, tag="msum")
            if fast:
                nc.scalar.activation(r, agg[:, 0:F], mybir.ActivationFunctionType.Relu,
                                     scale=rz, accum_out=msum)
            else:
                u = posb.tile([128, F], F32, tag="u")
                nc.vector.scalar_tensor_tensor(u, bB, zsrc, agg[:, 0:F],
                                               mybir.AluOpType.mult, mybir.AluOpType.add)
                nc.scalar.activation(r, u, mybir.ActivationFunctionType.Relu,
                                     scale=rz, accum_out=msum)
            scr = posb.tile([128, F], F32, tag="scr")
            ssq = posb.tile([128, 1], F32, tag="ssq")
            nc.scalar.activation(scr, r, mybir.ActivationFunctionType.Square,
                                 accum_out=ssq)
            vpe = posb.tile([128, 1], F32, tag="vpe")
            nc.vector._custom_dve(var_op, out=vpe, in0=ssq, in1=msum,
                                  s0=EPS, s1=1.0 / F, imm2=1.0 / F)
            mu = posb.tile([128, 1], F32, tag="mu")
            nc.vector.tensor_scalar(mu, msum, 1.0 / F, None, mybir.AluOpType.mult)
            sd = posb.tile([128, 1], F32, tag="sd")
            nc.scalar.activation(sd, vpe, mybir.ActivationFunctionType.Sqrt)
            rsd = posb.tile([128, 1], F32, tag="rsd")
            nc.vector.reciprocal_approx_fast(rsd, sd)

            if layer == 1:
                hb = posb.tile([128, F], BF16, tag="hb")
            else:
                hb = posb.tile([128, F2 + 1], BF16, tag="hb")
                nc.vector.memset(hb[:, F2:F2 + 1], 1.0)
            if fast:
                nc.vector._custom_dve(hb_op, out=hb[:, 0:F], in0=r, s0=mu, s1=rsd)
            else:
                h0 = posb.tile([128, F], F32, tag="h0")
                nc.vector._custom_dve(hb_op, out=h0, in0=r, s0=mu, s1=rsd)
                h1 = posb.tile([128, F], F32, tag="h1")
                nc.vector.tensor_tensor(h1, h0, gB, mybir.AluOpType.mult)
                nc.vector.tensor_tensor(hb[:, 0:F], h1, beB, mybir.AluOpType.add)

            if layer == 1:
                trp = aux_ps.tile([128, 128], BF16, tag="aux", name=f"trp{b}")
                nc.tensor.transpose(trp, hb, identb)
                lnbT = posb.tile([128, 128], BF16, tag="lnbT")
                nc.scalar.activation(lnbT, trp, mybir.ActivationFunctionType.Copy)
                proj = aux_ps.tile([128, F2 + 2], F32, tag="aux", name=f"proj{b}")
                nc.tensor.matmul(proj, lnbT, w2ext[0:128, :], start=True, stop=True)
                nc.scalar.activation(t2stage[:, b, 0:F2], proj[:, 0:F2],
                                     mybir.ActivationFunctionType.Copy)
                nc.vector.tensor_copy(sd2stage[:, b, :], proj[:, F2:F2 + 2])
            else:
                nc.tensor.matmul(pool_psum, gh_sb[:, b * G:(b + 1) * G], hb,
                                 start=(b == 0), stop=(b == NB - 1))

        agg_of = {}
        for g in groups:
            nlo_g, nhi_g = g["nlo"], g["nhi"]
            glo = gsb.tile([128, max(nlo_g, 1), ROW], BF16, tag="glo")
            ghi = gsb.tile([128, max(nhi_g, 1), ROW], BF16, tag="ghi")
            if nlo_g:
                nc.gpsimd.dma_gather(glo[:, 0:nlo_g, :], tlo_in,
                                     ilo_sb[:, g["lo0"] * 8:(g["lo0"] + nlo_g) * 8],
                                     nlo_g * 128, nlo_g * 128, ROW, single_packet=False)
            if nhi_g:
                nc.gpsimd.dma_gather(ghi[:, 0:nhi_g, :], thi_in,
                                     ihi_sb[:, g["hi0"] * 8:(g["hi0"] + nhi_g) * 8],
                                     nhi_g * 128, nhi_g * 128, ROW, single_packet=False)
            t0 = tiles.index(g["tiles"][0]) if g["tiles"] else 0

            for k, (b, half, spos) in enumerate(g["tiles"]):
                ti = t0 + k
                if half == "lo":
                    gt = glo[:, spos - g["lo0"], :]
                else:
                    gt = ghi[:, spos - g["hi0"], :]
                if b not in agg_of:
                    if layer == 1:
                        agg_of[b] = (agg_ps.tile([128, F], F32, tag="agg", name=f"agg{b}"),
                                     aggz_ps.tile([128, 1], F32, tag="aggz", name=f"aggz{b}"))
                    else:
                        agg_of[b] = (agg_ps.tile([128, F + 1], F32, tag="agg", name=f"agg{b}"),
                                     None)
                    ms = msb.tile([128, 128], BF16, tag="m", name=f"mself{b}")
                    nc.vector.tensor_scalar(ms, iota_b, iota_col, exs[:, b:b + 1],
                                            mybir.AluOpType.is_equal, mybir.AluOpType.mult)
                    at_, az_ = agg_of[b]
                    if layer == 1:
                        nc.tensor.matmul(at_, ms, town_sb[:, b, 0:F], start=True, stop=False)
                        nc.tensor.matmul(az_, ms, ones_col, start=True, stop=False)
                    else:
                        nc.tensor.matmul(at_, ms, town_sb[:, b, 0:F + 1], start=True, stop=False)
                m = msb.tile([128, 128], BF16, tag="m")
                nc.vector.tensor_scalar(m, iota_b, dl_sb[:, ti:ti + 1], ex[:, ti:ti + 1],
                                        mybir.AluOpType.is_equal, mybir.AluOpType.mult)
                aggt, aggzt = agg_of[b]
                if layer == 1:
                    nc.tensor.matmul(aggt, m, gt[:, 0:F],
                                     start=False, stop=(ti == last[b]))
                    nc.tensor.matmul(aggzt, m, ones_col,
                                     start=False, stop=(ti == last[b]))
                else:
                    nc.tensor.matmul(aggt, m, gt[:, 0:F + 1],
                                     start=False, stop=(ti == last[b]))
                if ti == last[b]:
                    a_, z_ = agg_of.pop(b)
                    postproc(b, a_, z_)
                    if layer == 1 and b == NB // 2 - 1:
                        nc.sync.dma_start(
                            t2_out.rearrange("(b p) c -> p b c", p=128)[:, 0:NB // 2, :],
                            t2stage[:, 0:NB // 2, :])
                        nc.sync.dma_start(
                            sd2_out.rearrange("(b p) c -> p b c", p=128)[:, 0:NB // 2, :],
                            sd2stage[:, 0:NB // 2, :])

        if layer == 1:
            t2ap = t2_out.rearrange("(b p) c -> p b c", p=128)
            sd2ap = sd2_out.rearrange("(b p) c -> p b c", p=128)
            nc.sync.dma_start(t2ap[:, NB // 2:, :], t2stage[:, NB // 2:, :])
            nc.sync.dma_start(sd2ap[:, NB // 2:, :], sd2stage[:, NB // 2:, :])
        else:
            pout = singles.tile([G, F2 + 1], F32)
            nc.vector.tensor_copy(pout, pool_psum)
            nc.sync.dma_start(pool_out, pout)
    nc.finalize()
    return nc


def _build_p3():
    nc = _new_nc()
    pin = nc.dram_tensor("pall", [G, NCORES * (F2 + 1)], F32, kind="ExternalInput").ap()
    wl_in = nc.dram_tensor("Wl", [F2, F2], F32, kind="ExternalInput").ap()
    bl_in = nc.dram_tensor("bl", [F2], F32, kind="ExternalInput").ap()
    wc_in = nc.dram_tensor("Wc", [1, F2], F32, kind="ExternalInput").ap()
    bc_in = nc.dram_tensor("bc", [1], F32, kind="ExternalInput").ap()
    id_in = nc.dram_tensor("ident", [128, 128], F32, kind="ExternalInput").ap()
    out = nc.dram_tensor("out", [G], F32, kind="ExternalOutput").ap()

    with tile.TileContext(nc, num_cores=NCORES) as tc, ExitStack() as ctx:
        singles = ctx.enter_context(tc.tile_pool(name="singles", bufs=1))
        ps = ctx.enter_context(tc.tile_pool(name="ps", bufs=4, space="PSUM"))

        ident = singles.tile([128, 128], F32)
        nc.sync.dma_start(ident, id_in)
        acc = singles.tile([G, (F2 + 1) * NCORES], F32)
        nc.sync.dma_start(acc, pin)
        tots = [singles.tile([G, F2 + 1], F32, tag=f"tot{i}", name=f"tot{i}") for i in range(NCORES - 1)]
        nc.vector.tensor_tensor(tots[0], acc[:, 0:F2 + 1], acc[:, F2 + 1:2 * (F2 + 1)],
                                mybir.AluOpType.add)
        for c in range(2, NCORES):
            nc.vector.tensor_tensor(tots[c - 1], tots[c - 2],
                                    acc[:, c * (F2 + 1):(c + 1) * (F2 + 1)],
                                    mybir.AluOpType.add)
        tot = tots[NCORES - 2]
        cnt = singles.tile([G, 1], F32)
        nc.vector.tensor_scalar(cnt, tot[:, F2:F2 + 1], 1.0, None, mybir.AluOpType.max)
        rc = singles.tile([G, 1], F32)
        nc.vector.reciprocal(rc, cnt)
        pm = singles.tile([G, F2], F32)
        nc.vector.tensor_scalar(pm, tot[:, 0:F2], rc, None, mybir.AluOpType.mult)
        pmT_ps = ps.tile([F2, G], F32, tag="ps")
        nc.tensor.transpose(pmT_ps, pm, ident[0:G, 0:G])
        pmT = singles.tile([F2, G], F32)
        nc.vector.tensor_copy(pmT, pmT_ps)

        wl_sb = singles.tile([F2, F2], F32)
        nc.sync.dma_start(wl_sb, wl_in)
        wlt_ps = ps.tile([F2, F2], F32, tag="ps")
        nc.tensor.transpose(wlt_ps, wl_sb, ident[0:F2, 0:F2])
        wlt = singles.tile([F2, F2], F32)
        nc.vector.tensor_copy(wlt, wlt_ps)
        bl_sb = singles.tile([F2, 1], F32)
        nc.sync.dma_start(bl_sb, bl_in.rearrange("(a b) -> a b", b=1))
        y1_ps = ps.tile([F2, G], F32, tag="ps")
        nc.tensor.matmul(y1_ps, wlt, pmT, start=True, stop=True)
        y1 = singles.tile([F2, G], F32)
        nc.scalar.activation(y1, y1_ps, mybir.ActivationFunctionType.Identity, bias=bl_sb)
        wc_sb = singles.tile([F2, 1], F32)
        nc.sync.dma_start(wc_sb, wc_in.rearrange("a b -> b a"))
        bc_sb = singles.tile([1, 1], F32)
        nc.sync.dma_start(bc_sb, bc_in.rearrange("(a b) -> a b", b=1))
        y2_ps = ps.tile([1, G], F32, tag="ps")
        nc.tensor.matmul(y2_ps, wc_sb, y1, start=True, stop=True)
        y2 = singles.tile([1, G], F32)
        nc.scalar.activation(y2, y2_ps, mybir.ActivationFunctionType.Identity, bias=bc_sb)
        nc.sync.dma_start(out.rearrange("(a b) -> a b", a=1), y2)
    nc.finalize()
    return nc


# --------------------------------------------------------------------------
# Entry point
# --------------------------------------------------------------------------

def _note(rr, name):
    global LAST_EXEC_NS
    ns = rr.exec_time_ns
    if ns is not None:
        EXEC_NS.append((name, ns, None))
        LAST_EXEC_NS += ns


def kernel(x, edge_index, batch, W1, a1_src, a1_dst, b1, g1, be1,
           W2, a2_src, a2_dst, b2, g2, be2, Wl, bl, Wc, bc):
    _register_ops()
    x = np.asarray(x, np.float32)
    edge_index = np.asarray(edge_index)
    batch = np.asarray(batch)
    W1 = np.asarray(W1, np.float32)
    W2 = np.asarray(W2, np.float32)
    b1 = np.asarray(b1, np.float32)
    g1 = np.asarray(g1, np.float32)
    be1 = np.asarray(be1, np.float32)
    b2 = np.asarray(b2, np.float32)
    g2 = np.asarray(g2, np.float32)
    be2 = np.asarray(be2, np.float32)
    ident = np.eye(128, dtype=np.float32)
    identb = np.eye(128, dtype=NPBF)

    fast1 = (not b1.any()) and np.all(g1 == 1.0) and (not be1.any())
    fast2 = (not b2.any()) and np.all(g2 == 1.0) and (not be2.any())

    plan = _prep_plan(edge_index)
    ghot = _prep_pool(batch)

    # host-side weight prep
    w1ext = np.concatenate([W1.T,
                            (W1.T @ np.asarray(a1_src, np.float32))[:, None],
                            (W1.T @ np.asarray(a1_dst, np.float32))[:, None]],
                           axis=1).astype(NPBF)                     # [128, 130]
    w2ext = np.concatenate([W2.T,
                            (W2.T @ np.asarray(a2_src, np.float32))[:, None],
                            (W2.T @ np.asarray(a2_dst, np.float32))[:, None]],
                           axis=1).astype(NPBF)                     # [128, 66]

    # ---- P0: table build -------------------------------------------------
    xT = np.zeros((NCORES, F1, SLP), NPBF)
    for c in range(NCORES):
        xT[c, :, :SL] = x[c * SL:(c + 1) * SL].T.astype(NPBF)
    nc0 = _build_p0()
    in0 = [{"xT": xT[c], "w1ext": w1ext} for c in range(NCORES)]
    _rr = _RUN(nc0, in0, core_ids=list(range(NCORES)), trace=TRACE)
    _note(_rr, "P0")
    r0 = _rr.results
    t1_full = np.concatenate([r0[c]["t1slice"] for c in range(NCORES)], axis=0)
    sd1_full = np.concatenate([r0[c]["sd1"] for c in range(NCORES)], axis=0)
    scol1, ccol1 = _edge_cols(plan, sd1_full[:, 0], sd1_full[:, 1])

    # ---- P1: layer 1 -----------------------------------------------------
    nc1 = _build_msg_layer(plan, 1, fast1)
    in1 = []
    for c in range(NCORES):
        d = {"tlo": t1_full[:LOROWS], "thi": t1_full[LOROWS:],
             "idxlo": plan["idx_lo"][c], "idxhi": plan["idx_hi"][c],
             "dstloc": plan["dstloc"][c], "scol": scol1[c], "ccol": ccol1[c],
             "town": t1_full[c * SLP:(c + 1) * SLP],
             "sself": sd1_full[c * SLP:(c + 1) * SLP, 0].reshape(NB, 128).T.copy(),
             "cself": sd1_full[c * SLP:(c + 1) * SLP, 1].reshape(NB, 128).T.copy(),
             "w2ext": w2ext, "identb": identb}
        if not fast1:
            d.update({"bias": b1, "gamma": g1, "beta": be1})
        in1.append(d)
    _rr = _RUN(nc1, in1, core_ids=list(range(NCORES)), trace=TRACE)
    _note(_rr, "P1")
    r1 = _rr.results
    t2_full = np.concatenate([r1[c]["t2slice"] for c in range(NCORES)], axis=0)
    sd2_full = np.concatenate([r1[c]["sd2"] for c in range(NCORES)], axis=0)
    scol2, ccol2 = _edge_cols(plan, sd2_full[:, 0], sd2_full[:, 1])

    # ---- P2: layer 2 + partial pool -------------------------------------
    nc2 = _build_msg_layer(plan, 2, fast2)
    in2 = []
    for c in range(NCORES):
        d = {"tlo": t2_full[:LOROWS], "thi": t2_full[LOROWS:],
             "idxlo": plan["idx_lo"][c], "idxhi": plan["idx_hi"][c],
             "dstloc": plan["dstloc"][c], "scol": scol2[c], "ccol": ccol2[c],
             "town": t2_full[c * SLP:(c + 1) * SLP],
             "sself": sd2_full[c * SLP:(c + 1) * SLP, 0].reshape(NB, 128).T.copy(),
             "cself": sd2_full[c * SLP:(c + 1) * SLP, 1].reshape(NB, 128).T.copy(),
             "ghot": ghot[c]}
        if not fast2:
            d.update({"bias": b2, "gamma": g2, "beta": be2})
        in2.append(d)
    _rr = _RUN(nc2, in2, core_ids=list(range(NCORES)), trace=TRACE)
    _note(_rr, "P2")
    r2 = _rr.results
    pall = np.stack([r2[c]["pooled"] for c in range(NCORES)], axis=0)
    pall = np.ascontiguousarray(pall.transpose(1, 0, 2).reshape(G, NCORES * (F2 + 1)))

    # ---- P3: combine + MLP ----------------------------------------------
    nc3 = _build_p3()
    in3 = [{"pall": pall, "Wl": np.asarray(Wl, np.float32),
            "bl": np.asarray(bl, np.float32), "Wc": np.asarray(Wc, np.float32),
            "bc": np.asarray(bc, np.float32), "ident": ident} for c in range(NCORES)]
    _rr = _RUN(nc3, in3, core_ids=list(range(NCORES)), trace=TRACE)
    _note(_rr, "P3")
    r3 = _rr.results
    return np.asarray(r3[0]["out"], np.float32)
